# revision 28
# baseline (speedup 1.0000x reference)
import math
from contextlib import ExitStack

import numpy as np

N, T, D, H = 512, 128, 512, 512
NC = 8
n = N // NC          # 64 samples per core
H4 = 4 * H           # 2048
SCALE = 1.0 / math.sqrt(H)

# QUANT: "bf16" or "fp8" (fp8 = DoubleRow for Wh/Wattn/Wx matmuls)
QUANT = "bf16"
S_A = 8.0 if QUANT == "fp8" else 1.0    # stationary (activations) scale
S_W = 16.0 if QUANT == "fp8" else 1.0   # weight scale
QS = 1.0 / (S_A * S_W)                  # gate descale

_cache = {}


def _build_kernel(has_bias):
    key = ("nc", has_bias)
    if key in _cache:
        return _cache[key]

    import concourse.bass as bass
    import concourse.tile as tile
    from concourse import bacc, mybir

    f32 = mybir.dt.float32
    bf16 = mybir.dt.bfloat16
    fp8 = mybir.dt.float8e4
    FP8 = QUANT == "fp8"
    qdt = fp8 if FP8 else bf16
    ALU = mybir.AluOpType
    ACTF = mybir.ActivationFunctionType
    AX = mybir.AxisListType
    DR = mybir.MatmulPerfMode.DoubleRow if FP8 else None
    # padded stationary tiles in fp8: [128, (4 kt, 128)] with data at
    # [kt, 64:128]; zeros at [kt, 0:64] (DoubleRow cannot use tile_position,
    # so rows 64:128 of psum are written via a zero-padded full-width lhsT)
    AW = 128 if FP8 else 64   # per-chunk stationary width

    nc = bacc.Bacc(
        "TRN2",
        target_bir_lowering=False,
        debug=False,
        enable_asserts=False,
        num_devices=NC,
    )

    xhd = nc.dram_tensor("xh", (T, 128, 256), qdt, kind="ExternalInput").ap()
    A2d = nc.dram_tensor("A2d", (128, 8 * H), bf16, kind="ExternalInput").ap()
    Wcd = nc.dram_tensor("Wc", (128, 12 * H4), qdt, kind="ExternalInput").ap()
    bd = nc.dram_tensor("bvec", (1, H4), bf16, kind="ExternalInput").ap()
    idd = nc.dram_tensor("identd", (n, n), bf16, kind="ExternalInput").ap()
    id2d = nc.dram_tensor("id2d", (128, n), bf16, kind="ExternalInput").ap()
    onesd = nc.dram_tensor("onesd", (1, n), bf16, kind="ExternalInput").ap()
    hs = nc.dram_tensor("hs", (T, n, H), bf16, kind="ExternalOutput").ap()

    with tile.TileContext(nc) as tc, ExitStack() as ctx:
        const_pool = ctx.enter_context(tc.tile_pool(name="const", bufs=1))
        gps_pool = ctx.enter_context(tc.tile_pool(name="gps", bufs=2, space="PSUM"))
        tp_pool = ctx.enter_context(tc.tile_pool(name="tp", bufs=3, space="PSUM"))
        aps_pool = ctx.enter_context(tc.tile_pool(name="aps", bufs=1, space="PSUM"))

        # ---- persistent tiles --------------------------------------------
        W_sb = const_pool.tile([128, 12 * H4], qdt)
        b_sb = const_pool.tile([1, H4], bf16)
        id_sb = const_pool.tile([n, n], bf16)    # x S_A (transpose identity)
        id2 = const_pool.tile([128, n], bf16)    # [I; I] plain
        ones_row = const_pool.tile([1, n], bf16)
        A2 = const_pool.tile([128, 8 * H], bf16)     # q=(ph,s); free=(pl, h)
        nc.sync.dma_start(W_sb[:], Wcd[:])
        nc.sync.dma_start(b_sb[:], bd[:])
        nc.sync.dma_start(id_sb[:], idd[:])
        nc.sync.dma_start(id2[:], id2d[:])
        nc.sync.dma_start(ones_row[:], onesd[:])
        nc.sync.dma_start(A2[:], A2d[:])

        h2 = const_pool.tile([128, H], bf16)         # h duplicated on both halves
        c_st = const_pool.tile([n, H], bf16)
        hT = const_pool.tile([128, 4 * AW], qdt)     # h^T chunks (x S_A)
        attnT = const_pool.tile([128, 4 * AW], qdt)  # attn^T chunks (x S_A)
        xqb = [const_pool.tile([128, 4 * AW], qdt, name=f"xqb{i}")
               for i in range(3)]
        if FP8:
            # zero the pad columns once; DMA/copies only touch data slots
            for tl in (hT, attnT, *xqb):
                nc.vector.memset(
                    tl[:].rearrange("q (t w) -> q t w", t=4)[:, :, 0:64], 0)

        # per-step scratch
        P = const_pool.tile([128, 8 * H], bf16)      # A2 * h products
        Q1 = const_pool.tile([128, 2048], bf16)      # h-fold tree levels
        Q2 = const_pool.tile([128, 1024], bf16)
        Q3 = const_pool.tile([128, 512], bf16)
        Q4 = const_pool.tile([128, 256], bf16)
        Q5 = const_pool.tile([128, 128], bf16)
        dot8 = const_pool.tile([128, 8], f32)
        th = const_pool.tile([128, 8], f32)
        thp1 = const_pool.tile([128, 8], bf16)
        om = const_pool.tile([128, 8], f32)
        rin = const_pool.tile([128, 8], f32)
        wexp = const_pool.tile([128, 8], f32)
        Dg = const_pool.tile([128, 8 * n], bf16)     # 8 diag(w_p-pair) tiles
        s8 = const_pool.tile([128, 1], f32)
        s8c = const_pool.tile([n, 1], f32)
        ssum = const_pool.tile([n, 1], f32)
        rs = const_pool.tile([n, 1], f32)
        attn_s = const_pool.tile([n, H], bf16)
        gi = const_pool.tile([n, H], bf16)
        gf = const_pool.tile([n, H], bf16)
        go = const_pool.tile([n, H], bf16)
        gg = const_pool.tile([n, H], bf16)
        t1 = const_pool.tile([n, H], bf16)
        t2 = const_pool.tile([n, H], bf16)
        tct = const_pool.tile([n, H], bf16)
        i0 = const_pool.tile([128, 4 * H], f32)      # h0 init scratch
        i1 = const_pool.tile([128, 2 * H], f32)
        h0h = const_pool.tile([n, H], f32)

        # gate psum: ps_if [128,512]: rows 0:64 = i, 64:128 = f (cols 0:512
        # of hbar are i, 512:1024 f); ps_og: rows 0:64 = o, 64:128 = g.
        def bank(pst, j):
            # returns (out_ap, zero_padded?, tile_position or None)
            ps_if, ps_og = pst[0], pst[1]
            ps = ps_if if j in (0, 1) else ps_og
            if j in (0, 2):
                return ps[0:n, :], False, (0, 0)
            if FP8:
                return ps[:, :], True, None
            return ps[n:128, :], False, (0, n)

        def mark_start(pst, j):
            # has_written clear is scoped to the partitions the MM writes, so
            # every quadrant must open with start=True. In fp8 the zero-padded
            # f/g matmuls span all 128 partitions and open the whole bank, so
            # the i/o quadrants must NOT re-clear (emitted after, see JORD).
            if j in pst[2]:
                return False
            pst[2].add(j)
            if FP8 and j in (0, 2):
                return False
            return True

        def mm_w(pst, m, j, lhsT8, stop):
            # weight matmul against matrix m (0=Wh, 1=Wattn, 2=Wx)
            out, zp, tpos = bank(pst, j)
            if FP8:
                lv = lhsT8[:].rearrange("q (t w) -> q t w", t=4)
                for cp in range(2):
                    rhs = W_sb[:, H4 * 4 * m + H4 * 2 * cp:
                               H4 * 4 * m + H4 * 2 * (cp + 1)].rearrange(
                        "q (t c) -> q t c", t=2)[:, :, 512 * j:512 * (j + 1)]
                    lhsT = lv[:, 2 * cp:2 * cp + 2, :] if zp else \
                        lv[:, 2 * cp:2 * cp + 2, 64:128]
                    nc.tensor.matmul(out, lhsT, rhs,
                                     start=(cp == 0 and mark_start(pst, j)),
                                     stop=(stop and cp == 1),
                                     perf_mode=DR,
                                     skip_group_check=True)
            else:
                for c in range(4):
                    rhs = W_sb[:, H4 * (4 * m + c) + 512 * j:
                               H4 * (4 * m + c) + 512 * (j + 1)]
                    lhsT = lhsT8[:, n * c:n * (c + 1)]
                    nc.tensor.matmul(out, lhsT, rhs,
                                     start=(c == 0 and mark_start(pst, j)),
                                     stop=(stop and c == 3),
                                     tile_position=tpos,
                                     skip_group_check=True)

        def mm_bias(pst, j):
            ps_if, ps_og = pst[0], pst[1]
            ps = ps_if if j in (0, 1) else ps_og
            out = ps[0:n, :] if j in (0, 2) else ps[n:128, :]
            tpos = (0, 0) if j in (0, 2) else (0, n)
            nc.tensor.matmul(out, ones_row[:], b_sb[:, 512 * j:512 * (j + 1)],
                             start=mark_start(pst, j), stop=False,
                             tile_position=tpos, skip_group_check=True)

        JORD = (3, 1, 2, 0) if FP8 else (3, 2, 0, 1)
        pss = {}

        def emit_biasx(t):
            ps_if = gps_pool.tile([128, 512], f32, tag="psif")
            ps_og = gps_pool.tile([128, 512], f32, tag="psog")
            pst = (ps_if, ps_og, set())
            pss[t] = pst
            for j in JORD:
                if has_bias:
                    mm_bias(pst, j)
                mm_w(pst, 2, j, xqb[t % 3], stop=False)
            return pst

        def emit_wh(t):
            for j in JORD:
                mm_w(pss[t], 0, j, hT, stop=False)

        def emit_hT():
            for ci in range(4):
                pt = tp_pool.tile([128, n], bf16)
                nc.tensor.transpose(pt[:], h2[0:n, 128 * ci:128 * (ci + 1)],
                                    id_sb[:])
                nc.scalar.copy(hT[:, AW * ci + AW - 64:AW * (ci + 1)], pt[:])

        def load_x(t):
            if FP8:
                dst = xqb[t % 3][:].rearrange("q (t w) -> q t w",
                                              t=4)[:, :, 64:128]
                nc.sync.dma_start(
                    dst, xhd[t].rearrange("q (t w) -> q t w", t=4))
            else:
                nc.sync.dma_start(xqb[t % 3][:], xhd[t])

        # ---- h0 = mean over p of A_flat; c0 = h0 -------------------------
        nc.vector.tensor_tensor(i0[:], A2[:, 0:2048], A2[:, 2048:4096], ALU.add)
        nc.vector.tensor_tensor(i1[:], i0[:, 0:1024], i0[:, 1024:2048], ALU.add)
        nc.vector.tensor_tensor(i0[:, 0:512], i1[:, 0:512], i1[:, 512:1024],
                                ALU.add)
        nc.vector.tensor_copy(i1[0:n, 0:512], i0[n:128, 0:512])
        nc.vector.tensor_tensor(h0h[:], i0[0:n, 0:512], i1[0:n, 0:512],
                                ALU.add)
        nc.scalar.activation(c_st[:], h0h[:], ACTF.Copy, scale=1.0 / 16.0)
        nc.scalar.activation(h2[0:n, :], h0h[:], ACTF.Copy, scale=1.0 / 16.0)
        nc.vector.tensor_copy(h2[n:128, :], h2[0:n, :])

        load_x(0)
        load_x(1)
        emit_hT()
        emit_biasx(0)
        emit_wh(0)

        Pv = P[:].rearrange("q (pl h) -> q pl h", pl=8)

        # ---- recurrence ---------------------------------------------------
        for t in range(T):
            pst = pss.pop(t)
            ps_if, ps_og = pst[0], pst[1]

            # attention dot: P = A2 * h2 (broadcast over pl), fold h by tree
            nc.vector.tensor_tensor(
                Pv, A2[:].rearrange("q (pl h) -> q pl h", pl=8),
                h2[:].rearrange("q (r h) -> q r h", r=1).broadcast_to(
                    [128, 8, H]),
                ALU.mult)
            nc.vector.tensor_tensor(
                Q1[:].rearrange("q (pl h) -> q pl h", pl=8),
                Pv[:, :, 0:256], Pv[:, :, 256:512], ALU.add)
            nc.vector.tensor_tensor(
                Q2[:].rearrange("q (pl h) -> q pl h", pl=8),
                Q1[:].rearrange("q (pl h) -> q pl h", pl=8)[:, :, 0:128],
                Q1[:].rearrange("q (pl h) -> q pl h", pl=8)[:, :, 128:256],
                ALU.add)
            nc.vector.tensor_tensor(
                Q3[:].rearrange("q (pl h) -> q pl h", pl=8),
                Q2[:].rearrange("q (pl h) -> q pl h", pl=8)[:, :, 0:64],
                Q2[:].rearrange("q (pl h) -> q pl h", pl=8)[:, :, 64:128],
                ALU.add)
            nc.vector.tensor_tensor(
                Q4[:].rearrange("q (pl h) -> q pl h", pl=8),
                Q3[:].rearrange("q (pl h) -> q pl h", pl=8)[:, :, 0:32],
                Q3[:].rearrange("q (pl h) -> q pl h", pl=8)[:, :, 32:64],
                ALU.add)
            nc.vector.tensor_tensor(
                Q5[:].rearrange("q (pl h) -> q pl h", pl=8),
                Q4[:].rearrange("q (pl h) -> q pl h", pl=8)[:, :, 0:16],
                Q4[:].rearrange("q (pl h) -> q pl h", pl=8)[:, :, 16:32],
                ALU.add)
            nc.vector.tensor_reduce(
                dot8[:], Q5[:].rearrange("q (pl h) -> q pl h", pl=8),
                axis=AX.X, op=ALU.add)

            # softmax via exp(x) = (1+tanh(x/2))/(1-tanh(x/2))
            nc.scalar.activation(th[:], dot8[:], ACTF.Tanh, scale=0.5 * SCALE)
            nc.scalar.activation(thp1[:], th[:], ACTF.Copy, bias=1.0)
            nc.scalar.activation(om[:], th[:], ACTF.Copy, bias=1.0, scale=-1.0)
            nc.vector.reciprocal(rin[:], om[:])
            nc.vector.tensor_tensor(wexp[:], thp1[:], rin[:], ALU.mult)
            nc.vector.tensor_reduce(s8[:], wexp[:], axis=AX.X, op=ALU.add)
            nc.vector.tensor_copy(s8c[:], s8[n:128, :])
            nc.vector.tensor_tensor(ssum[:], s8[0:n, :], s8c[:], ALU.add)
            nc.vector.reciprocal(rs[:], ssum[:])

            # weighted sum on PE: attn = sum_p w_p * A_p via stacked-diagonal
            for pl in range(8):
                eng = nc.vector if pl % 2 == 0 else nc.gpsimd
                eng.tensor_scalar(Dg[:, n * pl:n * (pl + 1)], id2[:],
                                  wexp[:, pl:pl + 1], None, ALU.mult)
            aps = aps_pool.tile([n, H], f32, tag="aps")
            for pl in range(8):
                nc.tensor.matmul(aps[:], Dg[:, n * pl:n * (pl + 1)],
                                 A2[:, 512 * pl:512 * (pl + 1)],
                                 start=(pl == 0), stop=(pl == 7),
                                 skip_group_check=True)
            # evict + normalize (scale = 1/sum)
            nc.scalar.activation(attn_s[:], aps[:], ACTF.Copy, scale=rs[:])

            # attn^T chunks (identity is pre-scaled by S_A)
            for ci in range(4):
                pt = tp_pool.tile([128, n], bf16)
                nc.tensor.transpose(pt[:], attn_s[:, 128 * ci:128 * (ci + 1)],
                                    id_sb[:])
                nc.scalar.copy(attnT[:, AW * ci + AW - 64:AW * (ci + 1)],
                               pt[:])

            # Wattn matmuls (close each bank)
            for j in JORD:
                mm_w(pst, 1, j, attnT, stop=(j in (2, 1)))

            # gates (j-ordered: g, o, i computed early; f last)
            gsc = {} if QS == 1.0 else {"scale": QS}
            nc.scalar.activation(gg[:], ps_og[n:128, :], ACTF.Tanh, **gsc)
            nc.scalar.activation(go[:], ps_og[0:n, :], ACTF.Sigmoid, **gsc)
            nc.scalar.activation(gi[:], ps_if[0:n, :], ACTF.Sigmoid, **gsc)
            nc.gpsimd.tensor_tensor(t2[:], gi[:], gg[:], ALU.mult)
            for hh in range(2):
                sl = slice(256 * hh, 256 * (hh + 1))
                nc.scalar.activation(gf[:, sl], ps_if[n:128, sl],
                                     ACTF.Sigmoid, **gsc)
                nc.vector.tensor_tensor(t1[:, sl], gf[:, sl], c_st[:, sl],
                                        ALU.mult)
                nc.vector.tensor_tensor(c_st[:, sl], t1[:, sl], t2[:, sl],
                                        ALU.add)
                nc.scalar.activation(tct[:, sl], c_st[:, sl], ACTF.Tanh)
                nc.vector.tensor_tensor(h2[0:n, sl], go[:, sl], tct[:, sl],
                                        ALU.mult)
            nc.vector.tensor_copy(h2[n:128, :], h2[0:n, :])

            nc.gpsimd.dma_start(hs[t], h2[0:n, :])

            # preload next step's PE work while gates tail runs (keeps PE warm)
            if t + 1 < T:
                emit_biasx(t + 1)
                emit_hT()
                emit_wh(t + 1)
            if t + 2 < T:
                load_x(t + 2)

    nc.compile()
    _cache[key] = nc
    return nc


LAST_RESULT = None


def kernel(x, A, Wx, Wh, Wattn, b):
    import os
    import ml_dtypes
    from concourse import bass_utils

    has_bias = bool(np.any(np.asarray(b)))
    nc = _build_kernel(has_bias)
    bft = ml_dtypes.bfloat16
    qt = ml_dtypes.float8_e4m3fn if QUANT == "fp8" else bft

    # W layout: 12 chunks of 128 rows: c 0-3 Wh, 4-7 Wattn, 8-11 Wx
    Wcat = np.concatenate([np.asarray(Wh), np.asarray(Wattn), np.asarray(Wx)],
                          axis=0)                         # (1536, 2048)
    Wc_host = np.ascontiguousarray(
        (Wcat * S_W).reshape(12, 128, H4).transpose(1, 0, 2).reshape(
            128, 12 * H4)).astype(qt)
    b_host = (np.asarray(b, dtype=np.float32) * (S_A * S_W)).reshape(
        1, H4).astype(bft)
    ident = (np.eye(n, dtype=np.float32) * S_A).astype(bft)
    id2_host = np.concatenate([np.eye(n), np.eye(n)], axis=0).astype(bft)
    ones_h = np.ones((1, n), dtype=bft)

    in_maps = []
    for k in range(NC):
        xc = np.asarray(x[n * k:n * (k + 1)], dtype=np.float32)   # (64, T, D)
        Ac = np.asarray(A[n * k:n * (k + 1)], dtype=np.float32)   # (64, H, 4, 4)
        xh_host = np.ascontiguousarray(
            (xc * S_A).transpose(1, 2, 0).reshape(T, 4, 128, n)
            .transpose(0, 2, 1, 3).reshape(T, 128, 4 * n)).astype(qt)
        A_flat = Ac.reshape(n, H, 16).transpose(0, 2, 1)          # (n, 16, H)
        A4 = A_flat.reshape(n, 2, 8, H)
        A2_host = np.ascontiguousarray(
            A4.transpose(1, 0, 2, 3).reshape(128, 8 * H)).astype(bft)
        in_maps.append({
            "xh": xh_host,
            "A2d": A2_host,
            "Wc": Wc_host,
            "bvec": b_host,
            "identd": ident,
            "id2d": id2_host,
            "onesd": ones_h,
        })

    trace = os.environ.get("KERNEL_TRACE") == "1"
    tmpdir = os.environ.get("KERNEL_TRACE_DIR") or None
    res = bass_utils.run_bass_kernel_spmd(
        nc, in_maps, core_ids=list(range(NC)), trace=trace, tmpdir=tmpdir
    )
    global LAST_RESULT
    LAST_RESULT = res

    out = np.empty((N, T, H), dtype=np.float32)
    for k in range(NC):
        hs_k = np.asarray(res.results[k]["hs"])           # (T, n, H) bf16
        out[n * k:n * (k + 1)] = hs_k.transpose(1, 0, 2).astype(np.float32)
    return out
